# revision 67
# baseline (speedup 1.0000x reference)
"""Multi-head attention (B=2, S=2048, D=1024, H=16) on 8 NeuronCores.

Sharding: batch x head-group tensor parallel. Core c owns batch c//4 and
heads [4*(c%4), 4*(c%4)+4) (a 256-col group of Wq/Wk/Wv and 256-row
group of Wo). Each core computes its (batch, head group) projections,
causal attention, and a partial output projection; the host sums the 4
partials per batch and adds bo. Halving both input rows (one batch) and
output rows per core halves HBM traffic vs pure head sharding.

Layout: everything transposed ([feature, seq]). Scores are computed as
scores^T [k, q] so softmax-exp feeds the PV matmul directly with k on
partitions. All matmuls run in 128-row mode: the K=64 per-head score
contraction is zero-padded to K=128 (per-head k tiles with the unused
64 partitions zeroed) so the PE never switches tiling modes (mode
switches drain the array). V is projected directly into [seq, depth]
layout (lhsT = x^T chunk) so no PE transposes are needed; a ones column
per head in the V tile makes the PV matmul emit softmax denominators.
The attention inner loop lags PV four blocks behind scores so the
ScalarE exp latency is fully hidden by later score matmuls and each
pass's first PV lands after the previous pass's normalize has released
the pv psum banks; projection / output-projection matmuls are
interleaved as fillers to keep the PE busy during exp latency.
"""

import os

import numpy as np
import ml_dtypes

B, S, D, H = 2, 2048, 1024, 16
DEPTH = D // H          # 64
N_CORES = 8
HP = 256                # per-core head-group width: 4 heads * 64
NHG = 2                 # 128-partition head subgroups per core (2 heads each)
SCALE = 1.0 / float(np.sqrt(DEPTH))
SC = 512                # q chunk (attention column chunk)
KB = 128                # k block (scores^T partition block)
N_DC = D // 128         # 8 contraction chunks for projections
N_SC = S // SC          # 4 q chunks
N_SB = S // 128         # 16 s blocks
NWARM = 40

MM_DTYPE = os.environ.get("KERNEL_MM_DTYPE", "bf16")

_CACHE = {}


def _np_dt():
    return ml_dtypes.bfloat16 if MM_DTYPE == "bf16" else np.float32


def _build():
    """Build + compile the per-core Bass program (same program, all cores)."""
    import concourse.bacc as bacc
    import concourse.mybir as mybir
    import concourse.tile as tile

    f32 = mybir.dt.float32
    dt = mybir.dt.bfloat16 if MM_DTYPE == "bf16" else mybir.dt.float32r
    Exp = mybir.ActivationFunctionType.Exp
    P = 128

    nc = bacc.Bacc("TRN2", target_bir_lowering=False, debug=False,
                   num_devices=N_CORES)

    xq = nc.dram_tensor("xq", [D, S], dt, kind="ExternalInput").ap()
    xk = nc.dram_tensor("xk", [D, S], dt, kind="ExternalInput").ap()
    xv = nc.dram_tensor("xv", [D, S], dt, kind="ExternalInput").ap()
    wq = nc.dram_tensor("wq", [D, HP], dt, kind="ExternalInput").ap()
    wk = nc.dram_tensor("wk", [D, HP], dt, kind="ExternalInput").ap()
    wv = nc.dram_tensor("wv", [D, HP], dt, kind="ExternalInput").ap()
    wo = nc.dram_tensor("wo", [HP, D], dt, kind="ExternalInput").ap()
    bq = nc.dram_tensor("bq", [HP], f32, kind="ExternalInput").ap()
    bk = nc.dram_tensor("bk", [HP], f32, kind="ExternalInput").ap()
    bv = nc.dram_tensor("bv", [HP], dt, kind="ExternalInput").ap()
    outp = nc.dram_tensor("outp", [S, D], dt, kind="ExternalOutput").ap()

    with tile.TileContext(nc) as tc:
        with (
            tc.tile_pool(name="wpool", bufs=1) as wpool,
            tc.tile_pool(name="xin", bufs=1) as xin,
            tc.tile_pool(name="pt", bufs=10) as pt_pool,
            tc.tile_pool(name="rc", bufs=2) as rc_pool,
            tc.tile_pool(name="ost", bufs=4) as ost_pool,
            tc.tile_pool(name="psc", bufs=2, space="PSUM") as psc_pool,
            tc.tile_pool(name="plong", bufs=2, space="PSUM") as plong,
            tc.tile_pool(name="pshort", bufs=2, space="PSUM") as pshort,
        ):
            # ---- warmup constant first so the PE can spin immediately ----
            zt = wpool.tile([P, P], dt, tag="zt")
            nc.vector.memset(zt[:, :], 0.0)

            # ---- weights / constants / inputs, DMA-ordered by first use:
            # wq, wk, then per q-chunk (q, k, v) input slices with wv/wo
            # slotted in just before their consumers need them ----
            w_sb = {}
            b_sb = {}
            for name, wdram, bdram in (("q", wq, bq), ("k", wk, bk)):
                wt = wpool.tile([P, N_DC, HP], dt, tag=f"w{name}", name=f"w{name}")
                nc.sync.dma_start(out=wt[:, :, :],
                                  in_=wdram.rearrange("(dc p) h -> p dc h", p=P))
                w_sb[name] = wt
                bt = wpool.tile([P, NHG], f32, tag=f"b{name}", name=f"b{name}")
                nc.sync.dma_start(out=bt[:, :],
                                  in_=bdram.rearrange("(hg p) -> p hg", p=P))
                b_sb[name] = bt

            xt = {}

            def dma_inputs(name, half):
                xdram = {"q": xq, "k": xk, "v": xv}[name]
                for dc in range(N_DC):
                    t = xin.tile([P, S // 2], dt, tag=f"x{name}{dc}{half}",
                                 name=f"x{name}{dc}{half}")
                    nc.sync.dma_start(
                        out=t[:, :],
                        in_=xdram[dc * P:(dc + 1) * P,
                                  half * (S // 2):(half + 1) * (S // 2)])
                    xt[(name, dc, half)] = t

            # ordered by first compute use; 2KB partition lines throughout
            dma_inputs("q", 0)
            dma_inputs("k", 0)
            wv_sb = wpool.tile([P, N_DC, HP], dt, tag="wv")
            nc.sync.dma_start(out=wv_sb[:, :, :],
                              in_=wv.rearrange("(dc p) h -> p dc h", p=P))
            bv_blk = wpool.tile([P, HP], dt, tag="bvblk")
            nc.vector.memset(bv_blk[:, :], 0.0)
            nc.sync.dma_start(out=bv_blk[0:1, :],
                              in_=bv.rearrange("(o h) -> o h", o=1))
            dma_inputs("v", 0)
            wo_sb = wpool.tile([P, NHG, D], dt, tag="wo")
            nc.sync.dma_start(out=wo_sb[:, :, :],
                              in_=wo.rearrange("(hg p) d -> p hg d", p=P))
            for name in ("q", "k", "v"):
                dma_inputs(name, 1)

            # zero-padded ones block (row 0 = 1): K=128 broadcast matmuls for
            # the v bias and softmax denominators
            onesP = wpool.tile([P, P], dt, tag="onesP")
            nc.vector.memset(onesP[:, :], 0.0)
            nc.vector.memset(onesP[0:1, :], 1.0)

            # persistent state tiles
            # xh_q: per head-subgroup [2 heads' depth = 128, S]
            xh_q = [wpool.tile([P, S], dt, tag=f"xhq{g}", name=f"xhq{g}")
                    for g in range(NHG)]
            # xh_k: per head, zero-padded so score matmuls run K=128:
            # head (g, hh) occupies partitions [hh*64, hh*64+64), rest 0
            xh_k = [wpool.tile([P, S], dt, tag=f"xhk{h}", name=f"xhk{h}")
                    for h in range(2 * NHG)]
            for h in range(2 * NHG):
                hh = h % 2
                nc.vector.memset(xh_k[h][(1 - hh) * 64:(2 - hh) * 64, :], 0.0)
            # vt: per subgroup [k 128, kb, 130]: cols 0:64 = head0 v,
            # col 64 = ones, 65:129 = head1 v, col 129 = ones
            vt = [wpool.tile([P, N_SB, 130], dt, tag=f"vt{g}", name=f"vt{g}")
                  for g in range(NHG)]
            for g in range(NHG):
                nc.vector.memset(vt[g][:, :, 64:65], 1.0)
                nc.vector.memset(vt[g][:, :, 129:130], 1.0)
            # attn2T: per subgroup [2 heads' depth = 128, S] normalized attn out^T
            attn2T = [wpool.tile([P, S], dt, tag=f"attn{g}", name=f"attn{g}")
                      for g in range(NHG)]
            # denominator staging: row 0 = denominators, rows 1:128 zero
            dsb = wpool.tile([P, 2 * SC], dt, tag="dsb")
            nc.vector.memset(dsb[:, :], 0.0)

            def xts(name, dc, sc):
                """[128, 512] view of input chunk (tensor, dc, q-chunk sc)."""
                half, off = divmod(sc, 2)
                return xt[(name, dc, half)][:, off * SC:(off + 1) * SC]

            # ---- PE warmup: keep HAM busy while first DMAs land ----
            warm_ps = pshort.tile([P, SC], f32, tag="pshort", name="warm")
            for wi in range(NWARM):
                nc.tensor.matmul(warm_ps[:, 0:P], lhsT=zt[:, :], rhs=zt[:, :],
                                 start=(wi == 0), stop=(wi == NWARM - 1))

            # ---- filler machinery ----
            filler = []

            def emit_fillers(n):
                for _ in range(n):
                    if not filler:
                        return
                    filler.pop(0)()

            def flush_fillers():
                while filler:
                    filler.pop(0)()

            # ---- projections for one q chunk ----
            def proj_qk_chain(name, g, sc):
                """One head-subgroup's q/k projection chain for chunk sc."""
                chain = pshort.tile([P, SC], f32, tag="pshort",
                                    name=f"ch_{name}{g}")
                for dc in range(N_DC):
                    nc.tensor.matmul(
                        chain[:, :],
                        lhsT=w_sb[name][:, dc, g * P:(g + 1) * P],
                        rhs=xts(name, dc, sc),
                        start=(dc == 0), stop=(dc == N_DC - 1))
                if name == "q":
                    nc.vector.tensor_scalar_add(
                        xh_q[g][:, sc * SC:(sc + 1) * SC], chain[:, :],
                        b_sb["q"][:, g:g + 1])
                else:
                    for hh in range(2):
                        h = 2 * g + hh
                        nc.vector.tensor_scalar_add(
                            xh_k[h][hh * 64:(hh + 1) * 64,
                                    sc * SC:(sc + 1) * SC],
                            chain[hh * 64:(hh + 1) * 64, :],
                            b_sb["k"][hh * 64:(hh + 1) * 64, g:g + 1])

            def proj_v_sb(sb):
                """Direct-layout V projection for s block sb: [s 128, 256]."""
                sc = sb // (SC // KB)
                vp = pshort.tile([P, SC], f32, tag="pshort", name="vp")
                for dc in range(N_DC):
                    nc.tensor.matmul(
                        vp[:, 0:HP],
                        lhsT=xts("v", dc, sc)[:, (sb % 4) * P:(sb % 4 + 1) * P],
                        rhs=wv_sb[:, dc, :],
                        start=(dc == 0), stop=False)
                nc.tensor.matmul(vp[:, 0:HP], lhsT=onesP[:, :], rhs=bv_blk[:, :],
                                 start=False, stop=True)
                for g in range(NHG):
                    # one strided copy per subgroup: head depths land at
                    # cols 0:64 and 65:129, skipping the ones columns
                    nc.vector.tensor_copy(
                        vt[g][:, sb, 0:130].rearrange("p (hh c) -> p hh c",
                                                      hh=2)[:, :, 0:64],
                        vp[:, g * P:(g + 1) * P].rearrange(
                            "p (hh c) -> p hh c", hh=2))

            def emit_proj_fillers(sc):
                for g in range(NHG):
                    filler.append(lambda g=g: proj_qk_chain("q", g, sc))
                    filler.append(lambda g=g: proj_qk_chain("k", g, sc))
                for sb in range(sc * 4, sc * 4 + 4):
                    filler.append(lambda sb=sb: proj_v_sb(sb))

            def outproj_sb(sb, nch, gsel=None):
                if outproj_sb.tail:
                    # attention psum is retired in the tail: borrow score
                    # banks so matmul pairs never wait on copy evacuation
                    po = psc_pool.tile([P, 2, SC], f32, tag="psc",
                                       name="po")[:, 0, :]
                else:
                    po = pshort.tile([P, SC], f32, tag="pshort", name="po")
                gs = range(NHG) if gsel is None else (gsel,)
                for i, g in enumerate(gs):
                    nc.tensor.matmul(
                        po[:, :],
                        lhsT=attn2T[g][:, sb * P:(sb + 1) * P],
                        rhs=wo_sb[:, g, nch * SC:(nch + 1) * SC],
                        start=(i == 0), stop=(i == len(gs) - 1))
                ost = ost_pool.tile([P, SC], dt, tag="ost")
                # PSUM evacuation on DVE while attention still needs ScalarE
                # for exp; in the tail (exp done) alternate engines so the
                # copies keep pace with the matmul pairs
                dst = outp[sb * P:(sb + 1) * P, nch * SC:(nch + 1) * SC]
                if outproj_sb.tail:
                    # ScalarE is idle in the tail and its queue is empty —
                    # the sync queue's backlog would delay these last writes
                    if nch == 1:
                        nc.scalar.copy(ost[:, :], po[:, :])
                    else:
                        nc.vector.tensor_copy(ost[:, :], po[:, :])
                    nc.scalar.dma_start(out=dst, in_=ost[:, :])
                else:
                    nc.vector.tensor_copy(ost[:, :], po[:, :])
                    nc.sync.dma_start(out=dst, in_=ost[:, :])
            outproj_sb.tail = False

            def emit_outproj_fillers(qc, gsel=None):
                for sb in range(qc * 4, qc * 4 + 4):
                    for nch in range(2):
                        filler.append(
                            lambda sb=sb, nch=nch: outproj_sb(sb, nch, gsel))

            # ---- attention pieces ----
            def qc_blocks(qc):
                nblk = SC // KB
                # off-diagonal (full width) first, then trimmed diagonal
                blocks = [(kb, SC, False) for kb in range(qc * nblk)]
                blocks += [(qc * nblk + j, SC - KB * j, True)
                           for j in range(nblk)]
                return blocks

            def scores_block(qc, g, kb, w, diag):
                """S matmuls + exp (+causal mask) for one 128-k block."""
                c0 = SC - w
                sc2 = psc_pool.tile([P, 2, SC], f32, tag="psc", name="sc2")
                for hh in range(2):
                    nc.tensor.matmul(
                        sc2[:, hh, c0:SC],
                        lhsT=xh_k[2 * g + hh][:, kb * KB:(kb + 1) * KB],
                        rhs=xh_q[g][:, qc * SC + c0:(qc + 1) * SC],
                        start=True, stop=True)
                pt = pt_pool.tile([P, 2, SC], dt, tag="pt")
                nc.scalar.activation(pt[:, :, 0:w], sc2[:, :, c0:SC],
                                     Exp, scale=SCALE)
                if diag:
                    nc.gpsimd.affine_select(
                        out=pt[:, :, 0:KB], in_=pt[:, :, 0:KB],
                        compare_op=mybir.AluOpType.is_ge,
                        fill=0.0, base=0,
                        pattern=[[0, 2], [1, KB]],
                        channel_multiplier=-1)
                return (pt, w, c0, kb)

            def pv_block(g, pvs, blk, first, last):
                pt, w, c0, kb = blk
                for hh in range(2):
                    nc.tensor.matmul(
                        pvs[hh][:, c0:SC],
                        lhsT=vt[g][:, kb, hh * 65:hh * 65 + 65],
                        rhs=pt[:, hh, 0:w],
                        start=first, stop=last)

            def normalize(qc, g, pvs):
                """Denominators sit in row 64 of each pv psum. Stage to dsb
                row 0, broadcast via zero-padded ones matmul, reciprocal,
                scale the numerators into attn2T."""
                for hh in range(2):
                    nc.vector.tensor_copy(dsb[0:1, hh * SC:(hh + 1) * SC],
                                          pvs[hh][64:65, :])
                bc_sb = rc_pool.tile([P, 2, SC], f32, tag="bc")
                for hh in range(2):
                    bc_ps = pshort.tile([P, SC], f32, tag="pshort",
                                        name="bc_ps")
                    nc.tensor.matmul(bc_ps[:, :], lhsT=onesP[:, :],
                                     rhs=dsb[:, hh * SC:(hh + 1) * SC],
                                     start=True, stop=True)
                    nc.vector.reciprocal_approx_fast(out=bc_sb[:, hh, :],
                                                     in_=bc_ps[:, :])
                qcols = slice(qc * SC, (qc + 1) * SC)
                for hh in range(2):
                    nc.vector.tensor_mul(
                        attn2T[g][hh * 64:(hh + 1) * 64, qcols],
                        pvs[hh][0:64, :], bc_sb[hh * 64:(hh + 1) * 64, hh, :])

            def attention(qc, g):
                """Combined pass: PV lags scores by two blocks so the exp of
                block i overlaps the score matmuls of blocks i+1 and i+2 —
                exp latency never paces the PE."""
                blocks = qc_blocks(qc)
                pvs = [plong.tile([65, SC], f32, tag="plong", name=f"pv{hh}")
                       for hh in range(2)]
                pending = []
                n = len(blocks)
                for i, (kb, w, diag) in enumerate(blocks):
                    pending.append((scores_block(qc, g, kb, w, diag), i))
                    if len(pending) > 4:
                        blk, bi = pending.pop(0)
                        pv_block(g, pvs, blk, bi == 0, False)
                    emit_fillers(1)
                for blk, bi in pending:
                    pv_block(g, pvs, blk, bi == 0, bi == n - 1)
                normalize(qc, g, pvs)

            # ---- main schedule ----
            # ramp: q/k projections for sc0+sc1 and qc0's scores+exp run
            # while the v inputs are still streaming; qc0's PV phase drains
            # once vt lands
            for g in range(NHG):
                proj_qk_chain("q", g, 0)
                proj_qk_chain("q", g, 1)
            for g in range(NHG):
                proj_qk_chain("k", g, 0)
            pts0 = [[scores_block(0, g, kb, w, diag)
                     for (kb, w, diag) in qc_blocks(0)] for g in range(NHG)]
            for g in range(NHG):
                proj_qk_chain("k", g, 1)
            for sb in range(4):
                proj_v_sb(sb)
            for g in range(NHG):
                pvs = [plong.tile([65, SC], f32, tag="plong", name=f"pv{hh}")
                       for hh in range(2)]
                for i, blk in enumerate(pts0[g]):
                    pv_block(g, pvs, blk, i == 0, i == len(pts0[g]) - 1)
                normalize(0, g, pvs)

            for sb in range(4, 8):
                filler.append(lambda sb=sb: proj_v_sb(sb))
            emit_outproj_fillers(0)
            emit_proj_fillers(2)

            for qc in range(1, N_SC):
                for g in range(NHG):
                    attention(qc, g)
                    # mid-qc: pull some pending work in at pass boundary
                    emit_fillers(2)
                # all projections for qc+1 must be emitted before its
                # attention reads xh_*; outproj for this qc becomes filler
                flush_fillers()
                emit_outproj_fillers(qc)
                if qc + 2 < N_SC:
                    emit_proj_fillers(qc + 2)
            outproj_sb.tail = True
            flush_fillers()

    nc.compile()
    return nc


def _get_program():
    if "nc" not in _CACHE:
        _CACHE["nc"] = _build()
    return _CACHE["nc"]


def _ensure_ntff_hook():
    """Install the axon NTFF profile hook (this image's antenv lacks
    axon_hooks, so run_bass_kernel_spmd(trace=True) would fail)."""
    import sys
    import types
    import ctypes
    import contextlib

    if "antenv.axon_hooks" in sys.modules:
        return
    import jax
    jax.devices()
    so_path = os.environ.get("PJRT_LIBRARY_PATH")
    mod = types.ModuleType("antenv.axon_hooks")
    state = {"hook": None}
    mod.set_axon_ntff_profile_hook = lambda h: state.__setitem__("hook", h)
    mod.get_axon_ntff_profile_hook = lambda: state["hook"]
    sys.modules["antenv.axon_hooks"] = mod
    if not so_path:
        return
    lib = ctypes.CDLL(so_path)
    if not hasattr(lib, "axon_start_nrt_profile"):
        return
    lib.axon_start_nrt_profile.argtypes = [
        ctypes.POINTER(ctypes.c_int64), ctypes.c_size_t,
    ]
    lib.axon_start_nrt_profile.restype = ctypes.c_int64
    lib.axon_stop_nrt_profile.argtypes = [ctypes.c_char_p]
    lib.axon_stop_nrt_profile.restype = ctypes.c_int64

    @contextlib.contextmanager
    def _hook(output_dir, device_ids):
        jax.devices()
        if device_ids:
            ids = (ctypes.c_int64 * len(device_ids))(*device_ids)
            rc = lib.axon_start_nrt_profile(ids, len(device_ids))
        else:
            rc = lib.axon_start_nrt_profile(None, 0)
        if rc != 0:
            raise RuntimeError(f"axon_start_nrt_profile rc={rc}")
        try:
            yield
        finally:
            n = lib.axon_stop_nrt_profile(str(output_dir).encode())
            print(f"ntff profile: {n} file(s) written to {output_dir}")

    state["hook"] = _hook


def kernel(q, k, v, mask, Wq, bq, Wk, bk, Wv, bv, Wo, bo, **_unused):
    from concourse import bass_utils

    nc = _get_program()
    npdt = _np_dt()

    q = np.asarray(q, dtype=np.float32)
    k = np.asarray(k, dtype=np.float32)
    v = np.asarray(v, dtype=np.float32)

    xqT = [np.ascontiguousarray(q[b].T.astype(npdt)) for b in range(B)]
    xkT = [np.ascontiguousarray(k[b].T.astype(npdt)) for b in range(B)]
    xvT = [np.ascontiguousarray(v[b].T.astype(npdt)) for b in range(B)]
    Wq = np.asarray(Wq, dtype=np.float32)
    Wk = np.asarray(Wk, dtype=np.float32)
    Wv = np.asarray(Wv, dtype=np.float32)
    Wo = np.asarray(Wo, dtype=np.float32)
    bq = np.asarray(bq, dtype=np.float32)
    bk = np.asarray(bk, dtype=np.float32)
    bv = np.asarray(bv, dtype=np.float32)
    bo = np.asarray(bo, dtype=np.float32)

    in_maps = []
    for c in range(N_CORES):
        b = c // 4
        hg = c % 4
        cs = slice(hg * HP, (hg + 1) * HP)
        in_maps.append({
            "xq": xqT[b], "xk": xkT[b], "xv": xvT[b],
            "wq": np.ascontiguousarray(Wq[:, cs].astype(npdt)),
            "wk": np.ascontiguousarray(Wk[:, cs].astype(npdt)),
            "wv": np.ascontiguousarray(Wv[:, cs].astype(npdt)),
            "wo": np.ascontiguousarray(Wo[cs, :].astype(npdt)),
            "bq": np.ascontiguousarray(bq[cs]),
            "bk": np.ascontiguousarray(bk[cs]),
            "bv": np.ascontiguousarray(bv[cs].astype(npdt)),
        })

    trace = bool(int(os.environ.get("KERNEL_TRACE", "0")))
    if trace:
        _ensure_ntff_hook()
    res = bass_utils.run_bass_kernel_spmd(
        nc, in_maps, core_ids=list(range(N_CORES)), trace=trace,
    )
    _CACHE["last_results"] = res

    out = np.zeros((B, S, D), dtype=np.float32)
    for c in range(N_CORES):
        out[c // 4] += np.asarray(res.results[c]["outp"], dtype=np.float32)
    out += bo[None, None, :]
    return out


# revision 71
# speedup vs baseline: 1.0038x; 1.0038x over previous
"""Multi-head attention (B=2, S=2048, D=1024, H=16) on 8 NeuronCores.

Sharding: batch x head-group tensor parallel. Core c owns batch c//4 and
heads [4*(c%4), 4*(c%4)+4) (a 256-col group of Wq/Wk/Wv and 256-row
group of Wo). Each core computes its (batch, head group) projections,
causal attention, and a partial output projection; the host sums the 4
partials per batch and adds bo. Halving both input rows (one batch) and
output rows per core halves HBM traffic vs pure head sharding.

Layout: everything transposed ([feature, seq]). Scores are computed as
scores^T [k, q] so softmax-exp feeds the PV matmul directly with k on
partitions. All matmuls run in 128-row mode: the K=64 per-head score
contraction is zero-padded to K=128 (per-head k tiles with the unused
64 partitions zeroed) so the PE never switches tiling modes (mode
switches drain the array). V is projected directly into [seq, depth]
layout (lhsT = x^T chunk) so no PE transposes are needed; a ones column
per head in the V tile makes the PV matmul emit softmax denominators.
The attention inner loop lags PV four blocks behind scores so the
ScalarE exp latency is fully hidden by later score matmuls and each
pass's first PV lands after the previous pass's normalize has released
the pv psum banks; projection / output-projection matmuls are
interleaved as fillers to keep the PE busy during exp latency.
"""

import os

import numpy as np
import ml_dtypes

B, S, D, H = 2, 2048, 1024, 16
DEPTH = D // H          # 64
N_CORES = 8
HP = 256                # per-core head-group width: 4 heads * 64
NHG = 2                 # 128-partition head subgroups per core (2 heads each)
SCALE = 1.0 / float(np.sqrt(DEPTH))
SC = 512                # q chunk (attention column chunk)
KB = 128                # k block (scores^T partition block)
N_DC = D // 128         # 8 contraction chunks for projections
N_SC = S // SC          # 4 q chunks
N_SB = S // 128         # 16 s blocks
NWARM = 40

MM_DTYPE = os.environ.get("KERNEL_MM_DTYPE", "bf16")

_CACHE = {}


def _np_dt():
    return ml_dtypes.bfloat16 if MM_DTYPE == "bf16" else np.float32


def _build():
    """Build + compile the per-core Bass program (same program, all cores)."""
    import concourse.bacc as bacc
    import concourse.mybir as mybir
    import concourse.tile as tile

    f32 = mybir.dt.float32
    dt = mybir.dt.bfloat16 if MM_DTYPE == "bf16" else mybir.dt.float32r
    Exp = mybir.ActivationFunctionType.Exp
    P = 128

    nc = bacc.Bacc("TRN2", target_bir_lowering=False, debug=False,
                   num_devices=N_CORES)

    # inputs arrive host-shuffled as [sc, dcpair, p, j, c]: each [128, 1024]
    # tile carries ONE 512-col q-chunk for TWO dc chunks side by side, so
    # q-chunk-granular streaming still gets full-bandwidth 2KB DMA lines
    xq = nc.dram_tensor("xq", [N_SC * N_DC // 2 * 128, 2 * SC], dt,
                        kind="ExternalInput").ap()
    xk = nc.dram_tensor("xk", [N_SC * N_DC // 2 * 128, 2 * SC], dt,
                        kind="ExternalInput").ap()
    xv = nc.dram_tensor("xv", [N_SC * N_DC // 2 * 128, 2 * SC], dt,
                        kind="ExternalInput").ap()
    wq = nc.dram_tensor("wq", [D, HP], dt, kind="ExternalInput").ap()
    wk = nc.dram_tensor("wk", [D, HP], dt, kind="ExternalInput").ap()
    wv = nc.dram_tensor("wv", [D, HP], dt, kind="ExternalInput").ap()
    wo = nc.dram_tensor("wo", [HP, D], dt, kind="ExternalInput").ap()
    bq = nc.dram_tensor("bq", [HP], f32, kind="ExternalInput").ap()
    bk = nc.dram_tensor("bk", [HP], f32, kind="ExternalInput").ap()
    bv = nc.dram_tensor("bv", [HP], dt, kind="ExternalInput").ap()
    outp = nc.dram_tensor("outp", [S, D], dt, kind="ExternalOutput").ap()

    with tile.TileContext(nc) as tc:
        with (
            tc.tile_pool(name="wpool", bufs=1) as wpool,
            tc.tile_pool(name="xin", bufs=1) as xin,
            tc.tile_pool(name="pt", bufs=10) as pt_pool,
            tc.tile_pool(name="rc", bufs=2) as rc_pool,
            tc.tile_pool(name="ost", bufs=4) as ost_pool,
            tc.tile_pool(name="psc", bufs=2, space="PSUM") as psc_pool,
            tc.tile_pool(name="plong", bufs=2, space="PSUM") as plong,
            tc.tile_pool(name="pshort", bufs=2, space="PSUM") as pshort,
        ):
            # ---- warmup constant first so the PE can spin immediately ----
            zt = wpool.tile([P, P], dt, tag="zt")
            nc.vector.memset(zt[:, :], 0.0)

            # ---- weights / constants / inputs, DMA-ordered by first use:
            # wq, wk, then per q-chunk (q, k, v) input slices with wv/wo
            # slotted in just before their consumers need them ----
            w_sb = {}
            b_sb = {}
            for name, wdram, bdram in (("q", wq, bq), ("k", wk, bk)):
                wt = wpool.tile([P, N_DC, HP], dt, tag=f"w{name}", name=f"w{name}")
                nc.sync.dma_start(out=wt[:, :, :],
                                  in_=wdram.rearrange("(dc p) h -> p dc h", p=P))
                w_sb[name] = wt
                bt = wpool.tile([P, NHG], f32, tag=f"b{name}", name=f"b{name}")
                nc.sync.dma_start(out=bt[:, :],
                                  in_=bdram.rearrange("(hg p) -> p hg", p=P))
                b_sb[name] = bt

            xt = {}

            def dma_inputs(name, sc):
                xdram = {"q": xq, "k": xk, "v": xv}[name]
                for dp in range(N_DC // 2):
                    t = xin.tile([P, 2, SC], dt, tag=f"x{name}{dp}{sc}",
                                 name=f"x{name}{dp}{sc}")
                    r0 = (sc * (N_DC // 2) + dp) * P
                    nc.sync.dma_start(
                        out=t[:, :, :],
                        in_=xdram[r0:r0 + P, :].rearrange(
                            "p (j c) -> p j c", j=2))
                    xt[(name, dp, sc)] = t

            # ordered by first compute use; 2KB partition lines throughout
            dma_inputs("q", 0)
            dma_inputs("k", 0)
            dma_inputs("q", 1)
            dma_inputs("k", 1)
            wv_sb = wpool.tile([P, N_DC, HP], dt, tag="wv")
            nc.sync.dma_start(out=wv_sb[:, :, :],
                              in_=wv.rearrange("(dc p) h -> p dc h", p=P))
            bv_blk = wpool.tile([P, HP], dt, tag="bvblk")
            nc.vector.memset(bv_blk[:, :], 0.0)
            nc.sync.dma_start(out=bv_blk[0:1, :],
                              in_=bv.rearrange("(o h) -> o h", o=1))
            dma_inputs("v", 0)
            dma_inputs("v", 1)
            wo_sb = wpool.tile([P, NHG, D], dt, tag="wo")
            nc.sync.dma_start(out=wo_sb[:, :, :],
                              in_=wo.rearrange("(hg p) d -> p hg d", p=P))
            for sc in range(2, N_SC):
                for name in ("q", "k", "v"):
                    dma_inputs(name, sc)

            # zero-padded ones block (row 0 = 1): K=128 broadcast matmuls for
            # the v bias and softmax denominators
            onesP = wpool.tile([P, P], dt, tag="onesP")
            nc.vector.memset(onesP[:, :], 0.0)
            nc.vector.memset(onesP[0:1, :], 1.0)

            # persistent state tiles
            # xh_q: per head-subgroup [2 heads' depth = 128, S]
            xh_q = [wpool.tile([P, S], dt, tag=f"xhq{g}", name=f"xhq{g}")
                    for g in range(NHG)]
            # xh_k: per head, zero-padded so score matmuls run K=128:
            # head (g, hh) occupies partitions [hh*64, hh*64+64), rest 0
            xh_k = [wpool.tile([P, S], dt, tag=f"xhk{h}", name=f"xhk{h}")
                    for h in range(2 * NHG)]
            for h in range(2 * NHG):
                hh = h % 2
                nc.vector.memset(xh_k[h][(1 - hh) * 64:(2 - hh) * 64, :], 0.0)
            # vt: per subgroup [k 128, kb, 130]: cols 0:64 = head0 v,
            # col 64 = ones, 65:129 = head1 v, col 129 = ones
            vt = [wpool.tile([P, N_SB, 130], dt, tag=f"vt{g}", name=f"vt{g}")
                  for g in range(NHG)]
            for g in range(NHG):
                nc.vector.memset(vt[g][:, :, 64:65], 1.0)
                nc.vector.memset(vt[g][:, :, 129:130], 1.0)
            # attn2T: per subgroup [2 heads' depth = 128, S] normalized attn out^T
            attn2T = [wpool.tile([P, S], dt, tag=f"attn{g}", name=f"attn{g}")
                      for g in range(NHG)]
            # denominator staging: row 0 = denominators, rows 1:128 zero
            dsb = wpool.tile([P, 2 * SC], dt, tag="dsb")
            nc.vector.memset(dsb[:, :], 0.0)

            def xts(name, dc, sc):
                """[128, 512] view of input chunk (tensor, dc, q-chunk sc)."""
                return xt[(name, dc // 2, sc)][:, dc % 2, :]

            # ---- PE warmup: keep HAM busy while first DMAs land ----
            warm_ps = pshort.tile([P, SC], f32, tag="pshort", name="warm")
            for wi in range(NWARM):
                nc.tensor.matmul(warm_ps[:, 0:P], lhsT=zt[:, :], rhs=zt[:, :],
                                 start=(wi == 0), stop=(wi == NWARM - 1))

            # ---- filler machinery ----
            filler = []

            def emit_fillers(n):
                for _ in range(n):
                    if not filler:
                        return
                    filler.pop(0)()

            def flush_fillers():
                while filler:
                    filler.pop(0)()

            # ---- projections for one q chunk ----
            def proj_qk_chain(name, g, sc):
                """One head-subgroup's q/k projection chain for chunk sc."""
                chain = pshort.tile([P, SC], f32, tag="pshort",
                                    name=f"ch_{name}{g}")
                for dc in range(N_DC):
                    nc.tensor.matmul(
                        chain[:, :],
                        lhsT=w_sb[name][:, dc, g * P:(g + 1) * P],
                        rhs=xts(name, dc, sc),
                        start=(dc == 0), stop=(dc == N_DC - 1))
                if name == "q":
                    nc.vector.tensor_scalar_add(
                        xh_q[g][:, sc * SC:(sc + 1) * SC], chain[:, :],
                        b_sb["q"][:, g:g + 1])
                else:
                    for hh in range(2):
                        h = 2 * g + hh
                        nc.vector.tensor_scalar_add(
                            xh_k[h][hh * 64:(hh + 1) * 64,
                                    sc * SC:(sc + 1) * SC],
                            chain[hh * 64:(hh + 1) * 64, :],
                            b_sb["k"][hh * 64:(hh + 1) * 64, g:g + 1])

            def proj_v_sb(sb):
                """Direct-layout V projection for s block sb: [s 128, 256]."""
                sc = sb // (SC // KB)
                vp = pshort.tile([P, SC], f32, tag="pshort", name="vp")
                for dc in range(N_DC):
                    nc.tensor.matmul(
                        vp[:, 0:HP],
                        lhsT=xts("v", dc, sc)[:, (sb % 4) * P:(sb % 4 + 1) * P],
                        rhs=wv_sb[:, dc, :],
                        start=(dc == 0), stop=False)
                nc.tensor.matmul(vp[:, 0:HP], lhsT=onesP[:, :], rhs=bv_blk[:, :],
                                 start=False, stop=True)
                for g in range(NHG):
                    # one strided copy per subgroup: head depths land at
                    # cols 0:64 and 65:129, skipping the ones columns
                    nc.vector.tensor_copy(
                        vt[g][:, sb, 0:130].rearrange("p (hh c) -> p hh c",
                                                      hh=2)[:, :, 0:64],
                        vp[:, g * P:(g + 1) * P].rearrange(
                            "p (hh c) -> p hh c", hh=2))

            def emit_proj_fillers(sc):
                for g in range(NHG):
                    filler.append(lambda g=g: proj_qk_chain("q", g, sc))
                    filler.append(lambda g=g: proj_qk_chain("k", g, sc))
                for sb in range(sc * 4, sc * 4 + 4):
                    filler.append(lambda sb=sb: proj_v_sb(sb))

            def outproj_sb(sb, nch, gsel=None):
                if outproj_sb.tail:
                    # attention psum is retired in the tail: borrow score
                    # banks so matmul pairs never wait on copy evacuation
                    po = psc_pool.tile([P, 2, SC], f32, tag="psc",
                                       name="po")[:, 0, :]
                else:
                    po = pshort.tile([P, SC], f32, tag="pshort", name="po")
                gs = range(NHG) if gsel is None else (gsel,)
                for i, g in enumerate(gs):
                    nc.tensor.matmul(
                        po[:, :],
                        lhsT=attn2T[g][:, sb * P:(sb + 1) * P],
                        rhs=wo_sb[:, g, nch * SC:(nch + 1) * SC],
                        start=(i == 0), stop=(i == len(gs) - 1))
                ost = ost_pool.tile([P, SC], dt, tag="ost")
                # PSUM evacuation on DVE while attention still needs ScalarE
                # for exp; in the tail (exp done) alternate engines so the
                # copies keep pace with the matmul pairs
                dst = outp[sb * P:(sb + 1) * P, nch * SC:(nch + 1) * SC]
                if outproj_sb.tail:
                    # ScalarE is idle in the tail and its queue is empty —
                    # the sync queue's backlog would delay these last writes
                    if nch == 1:
                        nc.scalar.copy(ost[:, :], po[:, :])
                    else:
                        nc.vector.tensor_copy(ost[:, :], po[:, :])
                    nc.scalar.dma_start(out=dst, in_=ost[:, :])
                else:
                    nc.vector.tensor_copy(ost[:, :], po[:, :])
                    nc.sync.dma_start(out=dst, in_=ost[:, :])
            outproj_sb.tail = False

            def emit_outproj_fillers(qc, gsel=None):
                for sb in range(qc * 4, qc * 4 + 4):
                    for nch in range(2):
                        filler.append(
                            lambda sb=sb, nch=nch: outproj_sb(sb, nch, gsel))

            # ---- attention pieces ----
            def qc_blocks(qc):
                nblk = SC // KB
                # off-diagonal (full width) first, then trimmed diagonal
                blocks = [(kb, SC, False) for kb in range(qc * nblk)]
                blocks += [(qc * nblk + j, SC - KB * j, True)
                           for j in range(nblk)]
                return blocks

            def scores_block(qc, g, kb, w, diag):
                """S matmuls + exp (+causal mask) for one 128-k block."""
                c0 = SC - w
                sc2 = psc_pool.tile([P, 2, SC], f32, tag="psc", name="sc2")
                for hh in range(2):
                    nc.tensor.matmul(
                        sc2[:, hh, c0:SC],
                        lhsT=xh_k[2 * g + hh][:, kb * KB:(kb + 1) * KB],
                        rhs=xh_q[g][:, qc * SC + c0:(qc + 1) * SC],
                        start=True, stop=True)
                pt = pt_pool.tile([P, 2, SC], dt, tag="pt")
                nc.scalar.activation(pt[:, :, 0:w], sc2[:, :, c0:SC],
                                     Exp, scale=SCALE)
                if diag:
                    nc.gpsimd.affine_select(
                        out=pt[:, :, 0:KB], in_=pt[:, :, 0:KB],
                        compare_op=mybir.AluOpType.is_ge,
                        fill=0.0, base=0,
                        pattern=[[0, 2], [1, KB]],
                        channel_multiplier=-1)
                return (pt, w, c0, kb)

            def pv_block(g, pvs, blk, first, last):
                pt, w, c0, kb = blk
                for hh in range(2):
                    nc.tensor.matmul(
                        pvs[hh][:, c0:SC],
                        lhsT=vt[g][:, kb, hh * 65:hh * 65 + 65],
                        rhs=pt[:, hh, 0:w],
                        start=first, stop=last)

            def normalize(qc, g, pvs):
                """Denominators sit in row 64 of each pv psum. Stage to dsb
                row 0, broadcast via zero-padded ones matmul, reciprocal,
                scale the numerators into attn2T."""
                for hh in range(2):
                    nc.vector.tensor_copy(dsb[0:1, hh * SC:(hh + 1) * SC],
                                          pvs[hh][64:65, :])
                bc_sb = rc_pool.tile([P, 2, SC], f32, tag="bc")
                for hh in range(2):
                    bc_ps = pshort.tile([P, SC], f32, tag="pshort",
                                        name="bc_ps")
                    nc.tensor.matmul(bc_ps[:, :], lhsT=onesP[:, :],
                                     rhs=dsb[:, hh * SC:(hh + 1) * SC],
                                     start=True, stop=True)
                    nc.vector.reciprocal_approx_fast(out=bc_sb[:, hh, :],
                                                     in_=bc_ps[:, :])
                qcols = slice(qc * SC, (qc + 1) * SC)
                for hh in range(2):
                    nc.vector.tensor_mul(
                        attn2T[g][hh * 64:(hh + 1) * 64, qcols],
                        pvs[hh][0:64, :], bc_sb[hh * 64:(hh + 1) * 64, hh, :])

            def attention(qc, g):
                """Combined pass: PV lags scores by two blocks so the exp of
                block i overlaps the score matmuls of blocks i+1 and i+2 —
                exp latency never paces the PE."""
                blocks = qc_blocks(qc)
                pvs = [plong.tile([65, SC], f32, tag="plong", name=f"pv{hh}")
                       for hh in range(2)]
                pending = []
                n = len(blocks)
                for i, (kb, w, diag) in enumerate(blocks):
                    pending.append((scores_block(qc, g, kb, w, diag), i))
                    if len(pending) > 4:
                        blk, bi = pending.pop(0)
                        pv_block(g, pvs, blk, bi == 0, False)
                    emit_fillers(1)
                for blk, bi in pending:
                    pv_block(g, pvs, blk, bi == 0, bi == n - 1)
                normalize(qc, g, pvs)

            # ---- main schedule ----
            # ramp: q/k projections for sc0+sc1 and qc0's scores+exp run
            # while the v inputs are still streaming; qc0's PV phase drains
            # once vt lands
            for g in range(NHG):
                proj_qk_chain("q", g, 0)
                proj_qk_chain("q", g, 1)
            for g in range(NHG):
                proj_qk_chain("k", g, 0)
            pts0 = [[scores_block(0, g, kb, w, diag)
                     for (kb, w, diag) in qc_blocks(0)] for g in range(NHG)]
            for g in range(NHG):
                proj_qk_chain("k", g, 1)
            for sb in range(4):
                proj_v_sb(sb)
            for g in range(NHG):
                pvs = [plong.tile([65, SC], f32, tag="plong", name=f"pv{hh}")
                       for hh in range(2)]
                for i, blk in enumerate(pts0[g]):
                    pv_block(g, pvs, blk, i == 0, i == len(pts0[g]) - 1)
                normalize(0, g, pvs)

            for sb in range(4, 8):
                filler.append(lambda sb=sb: proj_v_sb(sb))
            emit_outproj_fillers(0)
            emit_proj_fillers(2)

            for qc in range(1, N_SC):
                for g in range(NHG):
                    attention(qc, g)
                    # mid-qc: pull some pending work in at pass boundary
                    emit_fillers(2)
                # all projections for qc+1 must be emitted before its
                # attention reads xh_*; outproj for this qc becomes filler
                flush_fillers()
                emit_outproj_fillers(qc)
                if qc + 2 < N_SC:
                    emit_proj_fillers(qc + 2)
            outproj_sb.tail = True
            flush_fillers()

    nc.compile()
    return nc


def _get_program():
    if "nc" not in _CACHE:
        _CACHE["nc"] = _build()
    return _CACHE["nc"]


def _ensure_ntff_hook():
    """Install the axon NTFF profile hook (this image's antenv lacks
    axon_hooks, so run_bass_kernel_spmd(trace=True) would fail)."""
    import sys
    import types
    import ctypes
    import contextlib

    if "antenv.axon_hooks" in sys.modules:
        return
    import jax
    jax.devices()
    so_path = os.environ.get("PJRT_LIBRARY_PATH")
    mod = types.ModuleType("antenv.axon_hooks")
    state = {"hook": None}
    mod.set_axon_ntff_profile_hook = lambda h: state.__setitem__("hook", h)
    mod.get_axon_ntff_profile_hook = lambda: state["hook"]
    sys.modules["antenv.axon_hooks"] = mod
    if not so_path:
        return
    lib = ctypes.CDLL(so_path)
    if not hasattr(lib, "axon_start_nrt_profile"):
        return
    lib.axon_start_nrt_profile.argtypes = [
        ctypes.POINTER(ctypes.c_int64), ctypes.c_size_t,
    ]
    lib.axon_start_nrt_profile.restype = ctypes.c_int64
    lib.axon_stop_nrt_profile.argtypes = [ctypes.c_char_p]
    lib.axon_stop_nrt_profile.restype = ctypes.c_int64

    @contextlib.contextmanager
    def _hook(output_dir, device_ids):
        jax.devices()
        if device_ids:
            ids = (ctypes.c_int64 * len(device_ids))(*device_ids)
            rc = lib.axon_start_nrt_profile(ids, len(device_ids))
        else:
            rc = lib.axon_start_nrt_profile(None, 0)
        if rc != 0:
            raise RuntimeError(f"axon_start_nrt_profile rc={rc}")
        try:
            yield
        finally:
            n = lib.axon_stop_nrt_profile(str(output_dir).encode())
            print(f"ntff profile: {n} file(s) written to {output_dir}")

    state["hook"] = _hook


def kernel(q, k, v, mask, Wq, bq, Wk, bk, Wv, bv, Wo, bo, **_unused):
    from concourse import bass_utils

    nc = _get_program()
    npdt = _np_dt()

    q = np.asarray(q, dtype=np.float32)
    k = np.asarray(k, dtype=np.float32)
    v = np.asarray(v, dtype=np.float32)

    def shuffle(x, b):
        # [S, D] -> x^T [D, S] -> [sc, dcpair, p, (j, c)] so each (sc,
        # dcpair) DMA tile is [128, 1024] with 2KB contiguous lines
        xT = x[b].T.astype(npdt).reshape(N_DC // 2, 2, 128, N_SC, SC)
        return np.ascontiguousarray(
            xT.transpose(3, 0, 2, 1, 4).reshape(N_SC * N_DC // 2 * 128,
                                                2 * SC))

    xqT = [shuffle(q, b) for b in range(B)]
    xkT = [shuffle(k, b) for b in range(B)]
    xvT = [shuffle(v, b) for b in range(B)]
    Wq = np.asarray(Wq, dtype=np.float32)
    Wk = np.asarray(Wk, dtype=np.float32)
    Wv = np.asarray(Wv, dtype=np.float32)
    Wo = np.asarray(Wo, dtype=np.float32)
    bq = np.asarray(bq, dtype=np.float32)
    bk = np.asarray(bk, dtype=np.float32)
    bv = np.asarray(bv, dtype=np.float32)
    bo = np.asarray(bo, dtype=np.float32)

    in_maps = []
    for c in range(N_CORES):
        b = c // 4
        hg = c % 4
        cs = slice(hg * HP, (hg + 1) * HP)
        in_maps.append({
            "xq": xqT[b], "xk": xkT[b], "xv": xvT[b],
            "wq": np.ascontiguousarray(Wq[:, cs].astype(npdt)),
            "wk": np.ascontiguousarray(Wk[:, cs].astype(npdt)),
            "wv": np.ascontiguousarray(Wv[:, cs].astype(npdt)),
            "wo": np.ascontiguousarray(Wo[cs, :].astype(npdt)),
            "bq": np.ascontiguousarray(bq[cs]),
            "bk": np.ascontiguousarray(bk[cs]),
            "bv": np.ascontiguousarray(bv[cs].astype(npdt)),
        })

    trace = bool(int(os.environ.get("KERNEL_TRACE", "0")))
    if trace:
        _ensure_ntff_hook()
    res = bass_utils.run_bass_kernel_spmd(
        nc, in_maps, core_ids=list(range(N_CORES)), trace=trace,
    )
    _CACHE["last_results"] = res

    out = np.zeros((B, S, D), dtype=np.float32)
    for c in range(N_CORES):
        out[c // 4] += np.asarray(res.results[c]["outp"], dtype=np.float32)
    out += bo[None, None, :]
    return out
